# revision 40
# baseline (speedup 1.0000x reference)
"""Trainium2 Bass kernel for nn_Attention_2851858284976.

Dense transformer attention block, b=8 n=1024 dim=1024 heads=16.
Sharding: pure data parallel - one batch element per NeuronCore (8 cores).

Per-core math (batch element x of shape (n, dim)):
  Y = x @ w_qkv^T                              (n, 3*dim)
  Z = Y.reshape(49152, 64)   # raw reshape: rows are (token, col-block) pairs
  Q = Z[0:16384], K = Z[16384:32768], V = Z[32768:49152], each (16, 1024, 64)
  per head: P^T = exp(scale * K_h @ Q_h^T)     (softmax along the partition axis)
            [O^T; Zs*64] = [V_h | 1*64]^T @ P^T  (ones cols replicate the denom)
            oT_h = O^T * (1/Zs)
  out = (oT stacked).T @ w_out^T + b_out

Structure vs the previous version:
  - Phase 1 runs kt-OUTER over a ramp set of 8 PSUM banks so the first
    matmuls issue as soon as the first k-tile of xt/wqkvt lands (instead of
    waiting ~30us for the full 8MB input DMA).
  - DMA xbar transposes are issued from the sync queue (not ACT's queue).
  - No HAM warm-keeper matmuls in the attention loop.
  - exp reads/writes 2D contiguous APs (3D APs pay a per-outer-dim init).
  - The softmax denominator chain is one fused (64,2,512) copy/recip/mul
    per head instead of two per-ic chains.
  - exp activation table preloaded during phase 1.
"""
import numpy as np
import ml_dtypes

import concourse.bass as bass
import concourse.mybir as mybir
from concourse import bacc
from concourse.tile import TileContext
from concourse.bass_utils import run_bass_kernel_spmd

N_CORES = 8
N = 1024          # tokens
DIM = 1024
E3 = 3 * DIM      # qkv projection width
H = 16            # heads
HD = 64           # head dim
SCALE = HD ** -0.5
ZROWS = N * E3 // HD          # 49152 Z-rows, padded to 128 wide in DRAM

F32 = mybir.dt.float32
BF = mybir.dt.bfloat16
FT = mybir.ActivationFunctionType

# i-tile emission order: gets Q/K/V Z-row producers done in the order the
# attention heads consume them (Q:c0<-it0, K first chunks<-it2/3, V h0-3<-it5).
IT_ORDER = [0, 2, 3, 1, 4, 5, 6, 7]

EXP2D = True      # exp via 2D contiguous APs
WARM_N = 0        # HAM warm-keeper matmul width (0 = off)
OFFLOAD_JT = ()   # jt steps whose exp runs on DVE instead of ACT (net loss:
                  # DVE FIFO couples the exp chain with the denominator
                  # chains and the sps lifetime extension stalls scores)

# exp(s*SCALE) = (P4(s/128))^16 with P4 = deg-4 Taylor of e^u at u=s/128.
# |s| <= ~45 on this data -> |u| <= 0.35 -> approx rel err < 8e-4.
EXP_A = SCALE / 16.0
EXP_C1 = EXP_A
EXP_C2 = EXP_A * EXP_A / 2.0
EXP_C3 = EXP_A ** 3 / 6.0
EXP_C4 = EXP_A ** 4 / 24.0

_EXP_OPS = {}


def _register_exp_ops():
    """Register two custom DVE ops (poly-4 eval, ^16) used to offload part of
    the softmax exp from ACT to the otherwise-idle vector engine."""
    if _EXP_OPS:
        return _EXP_OPS
    import concourse.dve_ops as dve_ops
    from concourse.dve_spec import (
        Spec, Src0, C0, C1, C2, C3, One, lower, sq, _spill_c3_to_src1,
        _has_src1)
    from concourse.dve_uop import DveOpSpec

    def reg(name, spec):
        for op in dve_ops.OPS:
            if op.name == name:
                return op
        row = dve_ops._CUSTOM_DVE_ROW_BASE + len(dve_ops.OPS)
        shas = {}
        for ver in ("v3", "v4"):
            s = DveOpSpec(name=name, opcode=row, uops=lower(spec, ver=ver),
                          rd1_en=_has_src1(spec))
            shas[ver] = s.sha(ver)
        op = dve_ops.DveOp(name, spec, subdim=False, uops_sha=shas)
        dve_ops.OPS.append(op)
        dve_ops.CUSTOM_DVE_SPECS[name] = spec
        dve_ops._SUB_OPCODE_FOR_NAME[name] = row
        return op

    body = ((((C3 * Src0 + C2) * Src0 + C1) * Src0 + C0) * Src0 + One)
    p4 = reg("ANT_EXP_P4", Spec(
        body=_spill_c3_to_src1(body),
        reference=lambda in0, in1, s0, s1, imm2:
            ((((in1 * in0 + imm2) * in0 + s1) * in0 + s0) * in0 + 1.0)))
    p16 = reg("ANT_POW16", Spec(
        body=sq(sq(sq(sq(Src0)))),
        reference=lambda in0, in1, s0, s1, imm2: in0 ** 16))
    _EXP_OPS["p4"] = p4
    _EXP_OPS["p16"] = p16
    return _EXP_OPS


def build():
    nc = bacc.Bacc("TRN2", target_bir_lowering=False, num_devices=N_CORES)
    xt = nc.declare_dram_parameter("xt", [DIM, N], BF, isOutput=False)
    wqkvt = nc.declare_dram_parameter("wqkvt", [DIM, E3], BF, isOutput=False)
    woutt = nc.declare_dram_parameter("woutt", [DIM, DIM], BF, isOutput=False)
    bias = nc.declare_dram_parameter("bias", [1, DIM], F32, isOutput=False)
    outp = nc.declare_dram_parameter("out", [N, DIM], F32, isOutput=True)

    with TileContext(nc) as tc:
        with tc.tile_pool(name="dram", bufs=1, space="DRAM") as dpool, \
             tc.tile_pool(name="singles", bufs=1) as singles:
            zbuf = dpool.tile([ZROWS, 128], BF)    # Z rows, cols 64:128 unused
            zb3 = zbuf.rearrange("(r c) d -> r c d", c=48)   # (1024, 48, 128)

            # oT indexed [64*(h%2)+dd, h//2, ic, i] so the per-head denominator
            # mul and the phase-3 lhsT slices are both natural APs.
            oT = singles.tile([128, 8, 2, 512], BF)
            biasrep = singles.tile([128, DIM], F32)
            QKall = singles.tile([128, 32768], BF)
            WOT = singles.tile([128, 8, DIM], BF)

            ones_f = singles.tile([128, 8, HD], F32)
            nc.vector.memset(ones_f, 1.0)
            # [V | ones*64] stationary tiles for the PV matmul; ones half gives
            # the softmax denominator replicated on out rows 64-127.
            vh0 = singles.tile([128, 8, 2 * HD], BF)
            vh1 = singles.tile([128, 8, 2 * HD], BF)
            vh2 = singles.tile([128, 8, 2 * HD], BF)
            vh3 = singles.tile([128, 8, 2 * HD], BF)
            vhs = [vh0, vh1, vh2, vh3]
            for v in vhs:
                nc.vector.tensor_copy(v[:, :, HD:2 * HD], ones_f)

            # Preload the exp activation table while the PE ramps (saves the
            # ~2.7us table-load landing inside the attention window).
            expwarm = singles.tile([1, 8], F32)
            nc.vector.memset(expwarm, 0.0)
            nc.scalar.activation(expwarm, expwarm, FT.Exp, scale=1.0)

            exp_ops = _register_exp_ops() if OFFLOAD_JT else None
            if exp_ops:
                c4tile = singles.tile([128, 1], F32)
                nc.vector.memset(c4tile, EXP_C4)

            # ---------- phase 1: Y = x @ w_qkv^T -> zbuf ----------
            with tc.tile_pool(name="p1io", bufs=1) as p1io:
                XT = p1io.tile([128, 8, N], BF)
                WT = p1io.tile([128, 8, E3], BF)
                # kt-major input loads so the ramp set's kt=0 matmuls can
                # start after ~1MB of DMA instead of 8MB.
                for kt in range(8):
                    nc.sync.dma_start(
                        out=XT[:, kt, :], in_=xt[kt * 128:(kt + 1) * 128, :])
                    nc.sync.dma_start(
                        out=WT[:, kt, :], in_=wqkvt[kt * 128:(kt + 1) * 128, :])
                nc.sync.dma_start(
                    out=WOT, in_=woutt[:].rearrange("(a p) e -> p a e", p=128))
                nc.sync.dma_start(
                    out=biasrep, in_=bias[:].to_broadcast((128, DIM)))

                groups = [(it, ec) for it in IT_ORDER for ec in range(6)]
                # chunk c of the Q/K transpose covers Z-rows [4096c, 4096(c+1))
                # = Y rows [85.33c, 85.33(c+1)); fire it once the covering
                # i-tiles have written zbuf.
                need_tiles = {
                    c: set(range((4096 * c) // 48 // 128,
                                 ((4096 * (c + 1) - 1) // 48) // 128 + 1))
                    for c in range(8)
                }
                done_tiles = set()
                tr_fired = set()

                with tc.tile_pool(name="p1st", bufs=14) as p1st, \
                     tc.tile_pool(name="ps1", bufs=6, space="PSUM") as ps1:

                    def emit_group(it, ec, ps):
                        # copies on DVE: keeps ACT's queue empty so the first
                        # attention exp isn't stuck behind a copy backlog, and
                        # leaves 2 PSUM banks untouched by p1 so the attention
                        # pools allocate without waiting on the copy tail.
                        st = p1st.tile([128, 512], BF)
                        nc.vector.tensor_copy(st, ps)
                        nc.sync.dma_start(
                            out=zb3[it * 128:(it + 1) * 128,
                                    ec * 8:(ec + 1) * 8, 0:HD],
                            in_=st.rearrange("p (b d) -> p b d", d=HD))

                    def fire_transposes():
                        # transposes on ACT's queue, which is otherwise empty
                        # (p1 copies run on DVE): on sync they starve the zbuf
                        # writes -> st-slot WAR backlog -> PE gap -> HAM
                        # demotion; on a busy ACT queue they'd stall the exps.
                        # IT_ORDER puts all Q/K i-tiles first so every
                        # transpose completes before attention begins.
                        for c in range(8):
                            if c in tr_fired or not need_tiles[c] <= done_tiles:
                                continue
                            tr_fired.add(c)
                            nc.scalar.dma_start_transpose(
                                QKall[:, c * 4096:(c + 1) * 4096],
                                zbuf[c * 4096:(c + 1) * 4096, :])

                    # ramp set: kt-outer across 6 groups so PE work tracks the
                    # arriving k-tiles instead of waiting for all of them.
                    ramp = groups[:6]
                    rtiles = [ps1.tile([128, 512], F32, name=f"ramp{g}",
                                       tag="ps")
                              for g in range(len(ramp))]
                    for kt in range(8):
                        for g, (it, ec) in enumerate(ramp):
                            nc.tensor.matmul(
                                rtiles[g],
                                lhsT=XT[:, kt, it * 128:(it + 1) * 128],
                                rhs=WT[:, kt, ec * 512:(ec + 1) * 512],
                                start=(kt == 0), stop=(kt == 7),
                                skip_group_check=True)
                    for g, (it, ec) in enumerate(ramp):
                        emit_group(it, ec, rtiles[g])
                    done_tiles.add(IT_ORDER[0])
                    fire_transposes()

                    # steady state: kt-inner per group
                    for gi in range(6, len(groups)):
                        it, ec = groups[gi]
                        ps = ps1.tile([128, 512], F32, tag="ps")
                        for kt in range(8):
                            nc.tensor.matmul(
                                ps,
                                lhsT=XT[:, kt, it * 128:(it + 1) * 128],
                                rhs=WT[:, kt, ec * 512:(ec + 1) * 512],
                                start=(kt == 0), stop=(kt == 7))
                        emit_group(it, ec, ps)
                        if ec == 5:
                            done_tiles.add(it)
                            fire_transposes()
                            if it == 5:
                                # V rows for heads 0-3 live in i-tile 5; load
                                # them now so attention's first PV never waits.
                                # SWDGE queue: keeps sync clear for zbuf writes.
                                for h0 in range(3):
                                    nc.gpsimd.dma_start(
                                        out=vhs[h0][:, :, 0:HD],
                                        in_=zbuf[32768 + h0 * N:
                                                 32768 + (h0 + 1) * N,
                                                 0:HD].rearrange(
                                                     "(t p) d -> p t d", p=128))

            def qt_sl(h, lo, sz):
                return QKall[0:64, h * N + lo: h * N + lo + sz]

            def kt_sl(h, lo, sz):
                return QKall[0:64, 16384 + h * N + lo: 16384 + h * N + lo + sz]

            # ---------- phase 2: attention, software-pipelined ----
            # The scores+exp "produce" stream runs one (h, jt) step ahead of
            # the PV "consume" stream, across head boundaries. The window is
            # paced by ACT's exp throughput.
            with tc.tile_pool(name="pt", bufs=6) as ptpool, \
                 tc.tile_pool(name="rz", bufs=2) as rzpool, \
                 tc.tile_pool(name="mid", bufs=2) as midpool, \
                 tc.tile_pool(name="sps", bufs=2, space="PSUM") as spsum, \
                 tc.tile_pool(name="ops", bufs=2, space="PSUM") as opsum:
                steps = [(h, jt) for h in range(H) for jt in range(8)]

                def load_v(h):
                    nc.gpsimd.dma_start(
                        out=vhs[h % 4][:, :, 0:HD],
                        in_=zbuf[32768 + h * N: 32768 + (h + 1) * N,
                                 0:HD].rearrange("(t p) d -> p t d", p=128))

                def produce(h, jt):
                    sps = spsum.tile([128, 2, 512], F32, tag="sps")
                    if WARM_N:
                        # HAM warm-keeper: fills the PE's exp-wait slack so the
                        # activity monitor holds K=8/8. Overwritten by the real
                        # scores matmul (start=True).
                        nc.tensor.matmul(
                            sps[0:128, 0, 0:WARM_N],
                            lhsT=QKall[:, 0:128], rhs=QKall[:, 0:WARM_N],
                            start=True, stop=True)
                    for ic in range(2):
                        nc.tensor.matmul(
                            sps[:, ic, :],
                            lhsT=kt_sl(h, jt * 128, 128),
                            rhs=qt_sl(h, ic * 512, 512),
                            start=True, stop=True)
                    pt = ptpool.tile([128, 2, 512], BF, tag="pt")
                    if exp_ops and jt in OFFLOAD_JT:
                        # exp on DVE: (P4(s/128))^16 via two custom ops,
                        # relieving the ACT engine (the attention pacer).
                        mid = midpool.tile([128, 1024], F32, tag="mid")
                        nc.vector._custom_dve(
                            exp_ops["p4"], out=mid,
                            in0=sps.rearrange("p a b -> p (a b)"),
                            in1=c4tile, s0=EXP_C1, s1=EXP_C2, imm2=EXP_C3)
                        nc.vector._custom_dve(
                            exp_ops["p16"],
                            out=pt.rearrange("p a b -> p (a b)"), in0=mid)
                    elif EXP2D:
                        nc.scalar.activation(
                            pt.rearrange("p a b -> p (a b)"),
                            sps.rearrange("p a b -> p (a b)"),
                            FT.Exp, scale=SCALE)
                    else:
                        nc.scalar.activation(pt, sps, FT.Exp, scale=SCALE)
                    return pt

                # HAM promotion burst: ~4.5us of dependency-free back-to-back
                # matmuls. The p1 tail's stalls demote the PE clock to K=4/8;
                # promotion needs ~4us of continuous PE busy, which attention's
                # steady state never provides. This burst re-promotes once;
                # the per-step warm-keepers then hold K=8/8.
                burst = spsum.tile([128, 2, 512], F32, tag="sps")
                for _ in range(22):
                    nc.tensor.matmul(
                        burst[:, 0, :],
                        lhsT=QKall[:, 0:128], rhs=QKall[:, 0:512],
                        start=True, stop=True)

                ops = None
                deferred = []

                def consume(ops_t, pt_t, h, jt):
                    for ic in range(2):
                        nc.tensor.matmul(
                            ops_t[:, ic, :],
                            lhsT=vhs[h % 4][:, jt, :],
                            rhs=pt_t[:, ic, :],
                            start=(jt == 0), stop=(jt == 7),
                            skip_group_check=True)

                pt_next = produce(*steps[0])
                for s, (h, jt) in enumerate(steps):
                    po, hf = 64 * (h % 2), h // 2
                    if jt == 0:
                        ops = opsum.tile([128, 2, 512], F32, tag="ops")
                    ops_cur = ops
                    pt_cur = pt_next
                    if s + 1 < len(steps):
                        pt_next = produce(*steps[s + 1])
                    # DVE-offloaded steps' PV is deferred 2 steps so the PE's
                    # in-order queue never waits on the slower DVE exp chain.
                    deferred = [(d - 1, o, p, hh, jj)
                                for d, o, p, hh, jj in deferred]
                    while deferred and (deferred[0][0] <= 0 or jt == 7):
                        _, o, p, hh, jj = deferred.pop(0)
                        consume(o, p, hh, jj)
                    if exp_ops and jt in OFFLOAD_JT:
                        deferred.append((2, ops_cur, pt_cur, h, jt))
                    else:
                        consume(ops_cur, pt_cur, h, jt)
                    if jt == 5 and h + 3 < H:
                        load_v(h + 3)
                    if jt == 7:
                        # custom-DVE reciprocal can't read PSUM; stage the
                        # denominator through SBUF. One fused chain per head.
                        zst = rzpool.tile([64, 2, 512], F32, tag="zst")
                        nc.vector.tensor_copy(zst, ops_cur[64:128, :, :])
                        rzs = rzpool.tile([64, 2, 512], F32, tag="rzs")
                        nc.vector.reciprocal_approx_fast(rzs, zst)
                        nc.vector.tensor_mul(
                            oT[po:po + 64, hf, :, :],
                            ops_cur[0:64, :, :], rzs)

            # ---------- phase 3: out = oT.T @ w_out^T + b ----------
            with tc.tile_pool(name="p3st", bufs=4) as p3st, \
                 tc.tile_pool(name="ps3", bufs=4, space="PSUM") as ps3:
                for it in range(8):
                    for ec in range(2):
                        rps = ps3.tile([128, 512], F32)
                        for ct in range(8):
                            nc.tensor.matmul(
                                rps,
                                lhsT=oT[:, ct, it // 4,
                                        (it % 4) * 128:(it % 4 + 1) * 128],
                                rhs=WOT[:, ct, ec * 512:(ec + 1) * 512],
                                start=(ct == 0), stop=(ct == 7))
                        ost = p3st.tile([128, 512], F32)
                        nc.vector.tensor_add(
                            ost, rps, biasrep[:, ec * 512:(ec + 1) * 512])
                        nc.sync.dma_start(
                            out=outp[it * 128:(it + 1) * 128,
                                     ec * 512:(ec + 1) * 512],
                            in_=ost)

    nc.finalize()
    return nc


_CACHE = {}


def _get_nc():
    if "nc" not in _CACHE:
        _CACHE["nc"] = build()
    return _CACHE["nc"]


def make_in_maps(x, w_qkv, w_out, b_out):
    bf = ml_dtypes.bfloat16
    wqkvt = np.ascontiguousarray(np.asarray(w_qkv, dtype=np.float32).T).astype(bf)
    woutt = np.ascontiguousarray(np.asarray(w_out, dtype=np.float32).T).astype(bf)
    bias = np.ascontiguousarray(np.asarray(b_out, dtype=np.float32).reshape(1, DIM))
    x = np.asarray(x, dtype=np.float32)
    return [
        {
            "xt": np.ascontiguousarray(x[b].T).astype(bf),
            "wqkvt": wqkvt,
            "woutt": woutt,
            "bias": bias,
        }
        for b in range(N_CORES)
    ]


def kernel(x, w_qkv, w_out, b_out):
    nc = _get_nc()
    in_maps = make_in_maps(x, w_qkv, w_out, b_out)
    res = run_bass_kernel_spmd(nc, in_maps, core_ids=list(range(N_CORES)))
    return np.stack(
        [res.results[b]["out"] for b in range(N_CORES)], axis=0
    ).astype(np.float32)


# revision 43
# speedup vs baseline: 1.1718x; 1.1718x over previous
"""Trainium2 Bass kernel for nn_Attention_2851858284976.

Dense transformer attention block, b=8 n=1024 dim=1024 heads=16.
Sharding: pure data parallel - one batch element per NeuronCore (8 cores).

Per-core math (batch element x of shape (n, dim)):
  Y = x @ w_qkv^T                              (n, 3*dim)
  Z = Y.reshape(49152, 64)   # raw reshape: rows are (token, col-block) pairs
  Q = Z[0:16384], K = Z[16384:32768], V = Z[32768:49152], each (16, 1024, 64)
  per head: P^T = exp(scale * K_h @ Q_h^T)     (softmax along the partition axis)
            [O^T; Zs*64] = [V_h | 1*64]^T @ P^T  (ones cols replicate the denom)
            oT_h = O^T * (1/Zs)
  out = (oT stacked).T @ w_out^T + b_out

Structure vs the previous version:
  - Phase 1 runs kt-OUTER over a ramp set of 8 PSUM banks so the first
    matmuls issue as soon as the first k-tile of xt/wqkvt lands (instead of
    waiting ~30us for the full 8MB input DMA).
  - DMA xbar transposes are issued from the sync queue (not ACT's queue).
  - No HAM warm-keeper matmuls in the attention loop.
  - exp reads/writes 2D contiguous APs (3D APs pay a per-outer-dim init).
  - The softmax denominator chain is one fused (64,2,512) copy/recip/mul
    per head instead of two per-ic chains.
  - exp activation table preloaded during phase 1.
"""
import numpy as np
import ml_dtypes

import concourse.bass as bass
import concourse.mybir as mybir
from concourse import bacc
from concourse.tile import TileContext
from concourse.bass_utils import run_bass_kernel_spmd

N_CORES = 8
N = 1024          # tokens
DIM = 1024
E3 = 3 * DIM      # qkv projection width
H = 16            # heads
HD = 64           # head dim
SCALE = HD ** -0.5
ZROWS = N * E3 // HD          # 49152 Z-rows, padded to 128 wide in DRAM

F32 = mybir.dt.float32
BF = mybir.dt.bfloat16
FT = mybir.ActivationFunctionType

# i-tile emission order: gets Q/K/V Z-row producers done in the order the
# attention heads consume them (Q:c0<-it0, K first chunks<-it2/3, V h0-3<-it5).
IT_ORDER = [0, 2, 3, 1, 4, 5, 6, 7]

EXP2D = True      # exp via 2D contiguous APs
WARM_N = 0        # HAM warm-keeper matmul width (0 = off)
OFFLOAD_JT = ()   # jt steps whose exp runs on DVE instead of ACT (net loss:
                  # DVE FIFO couples the exp chain with the denominator
                  # chains and the sps lifetime extension stalls scores)

# exp(s*SCALE) = (P4(s/128))^16 with P4 = deg-4 Taylor of e^u at u=s/128.
# |s| <= ~45 on this data -> |u| <= 0.35 -> approx rel err < 8e-4.
EXP_A = SCALE / 16.0
EXP_C1 = EXP_A
EXP_C2 = EXP_A * EXP_A / 2.0
EXP_C3 = EXP_A ** 3 / 6.0
EXP_C4 = EXP_A ** 4 / 24.0

_EXP_OPS = {}


def _register_exp_ops():
    """Register two custom DVE ops (poly-4 eval, ^16) used to offload part of
    the softmax exp from ACT to the otherwise-idle vector engine."""
    if _EXP_OPS:
        return _EXP_OPS
    import concourse.dve_ops as dve_ops
    from concourse.dve_spec import (
        Spec, Src0, C0, C1, C2, C3, One, lower, sq, _spill_c3_to_src1,
        _has_src1)
    from concourse.dve_uop import DveOpSpec

    def reg(name, spec):
        for op in dve_ops.OPS:
            if op.name == name:
                return op
        row = dve_ops._CUSTOM_DVE_ROW_BASE + len(dve_ops.OPS)
        shas = {}
        for ver in ("v3", "v4"):
            s = DveOpSpec(name=name, opcode=row, uops=lower(spec, ver=ver),
                          rd1_en=_has_src1(spec))
            shas[ver] = s.sha(ver)
        op = dve_ops.DveOp(name, spec, subdim=False, uops_sha=shas)
        dve_ops.OPS.append(op)
        dve_ops.CUSTOM_DVE_SPECS[name] = spec
        dve_ops._SUB_OPCODE_FOR_NAME[name] = row
        return op

    body = ((((C3 * Src0 + C2) * Src0 + C1) * Src0 + C0) * Src0 + One)
    p4 = reg("ANT_EXP_P4", Spec(
        body=_spill_c3_to_src1(body),
        reference=lambda in0, in1, s0, s1, imm2:
            ((((in1 * in0 + imm2) * in0 + s1) * in0 + s0) * in0 + 1.0)))
    p16 = reg("ANT_POW16", Spec(
        body=sq(sq(sq(sq(Src0)))),
        reference=lambda in0, in1, s0, s1, imm2: in0 ** 16))
    _EXP_OPS["p4"] = p4
    _EXP_OPS["p16"] = p16
    return _EXP_OPS


def build():
    nc = bacc.Bacc("TRN2", target_bir_lowering=False, num_devices=N_CORES)
    xt = nc.declare_dram_parameter("xt", [DIM, N], BF, isOutput=False)
    wqkvt = nc.declare_dram_parameter("wqkvt", [DIM, E3], BF, isOutput=False)
    woutt = nc.declare_dram_parameter("woutt", [DIM, DIM], BF, isOutput=False)
    bias = nc.declare_dram_parameter("bias", [1, DIM], F32, isOutput=False)
    outp = nc.declare_dram_parameter("out", [N, DIM], F32, isOutput=True)

    with TileContext(nc) as tc:
        with tc.tile_pool(name="dram", bufs=1, space="DRAM") as dpool, \
             tc.tile_pool(name="singles", bufs=1) as singles:
            zbuf = dpool.tile([ZROWS, 128], BF)    # Z rows, cols 64:128 unused
            zb3 = zbuf.rearrange("(r c) d -> r c d", c=48)   # (1024, 48, 128)

            # oT indexed [64*(h%2)+dd, h//2, ic, i] so the per-head denominator
            # mul and the phase-3 lhsT slices are both natural APs.
            oT = singles.tile([128, 8, 2, 512], BF)
            biasrep = singles.tile([128, DIM], F32)
            QKall = singles.tile([128, 32768], BF)
            WOT = singles.tile([128, 8, DIM], BF)

            ones_f = singles.tile([128, 8, HD], F32)
            nc.vector.memset(ones_f, 1.0)
            # [V | ones*64] stationary tiles for the PV matmul; ones half gives
            # the softmax denominator replicated on out rows 64-127.
            vh0 = singles.tile([128, 8, 2 * HD], BF)
            vh1 = singles.tile([128, 8, 2 * HD], BF)
            vh2 = singles.tile([128, 8, 2 * HD], BF)
            vh3 = singles.tile([128, 8, 2 * HD], BF)
            vhs = [vh0, vh1, vh2, vh3]
            for v in vhs:
                nc.vector.tensor_copy(v[:, :, HD:2 * HD], ones_f)

            # Preload the exp activation table while the PE ramps (saves the
            # ~2.7us table-load landing inside the attention window).
            expwarm = singles.tile([1, 8], F32)
            nc.vector.memset(expwarm, 0.0)
            nc.scalar.activation(expwarm, expwarm, FT.Exp, scale=1.0)

            exp_ops = _register_exp_ops() if OFFLOAD_JT else None
            if exp_ops:
                c4tile = singles.tile([128, 1], F32)
                nc.vector.memset(c4tile, EXP_C4)

            # ---------- phase 1: Y = x @ w_qkv^T -> zbuf ----------
            with tc.tile_pool(name="p1io", bufs=1) as p1io:
                XT = p1io.tile([128, 8, N], BF)
                WT = p1io.tile([128, 8, E3], BF)
                # kt-major input loads so the ramp set's kt=0 matmuls can
                # start after ~1MB of DMA instead of 8MB.
                for kt in range(8):
                    nc.sync.dma_start(
                        out=XT[:, kt, :], in_=xt[kt * 128:(kt + 1) * 128, :])
                    nc.sync.dma_start(
                        out=WT[:, kt, :], in_=wqkvt[kt * 128:(kt + 1) * 128, :])
                nc.sync.dma_start(
                    out=WOT, in_=woutt[:].rearrange("(a p) e -> p a e", p=128))
                nc.sync.dma_start(
                    out=biasrep, in_=bias[:].to_broadcast((128, DIM)))

                groups = [(it, ec) for it in IT_ORDER for ec in range(6)]
                # transpose chunk c covers Z-rows [2048c, 2048(c+1)) = Y rows
                # [42.67c, 42.67(c+1)); fire it once the covering i-tiles have
                # written zbuf. 2048-row chunks (vs 4096) halve each xbar
                # burst so the DRAM reads interleave with the zbuf writes
                # instead of stalling them (st-slot WAR -> PE gap -> HAM).
                NTR = 16
                TRROWS = ZROWS * 2 // 3 // NTR
                need_tiles = {
                    c: set(range((TRROWS * c) // 48 // 128,
                                 ((TRROWS * (c + 1) - 1) // 48) // 128 + 1))
                    for c in range(NTR)
                }
                done_tiles = set()
                tr_fired = set()

                with tc.tile_pool(name="p1st", bufs=8) as p1st, \
                     tc.tile_pool(name="ps1", bufs=6, space="PSUM") as ps1:

                    def emit_group(it, ec, ps):
                        # copies on DVE: keeps ACT's queue empty so the first
                        # attention exp isn't stuck behind a copy backlog, and
                        # leaves 2 PSUM banks untouched by p1 so the attention
                        # pools allocate without waiting on the copy tail.
                        st = p1st.tile([128, 512], BF)
                        nc.vector.tensor_copy(st, ps)
                        nc.sync.dma_start(
                            out=zb3[it * 128:(it + 1) * 128,
                                    ec * 8:(ec + 1) * 8, 0:HD],
                            in_=st.rearrange("p (b d) -> p b d", d=HD))

                    def fire_transposes():
                        # transposes on ACT's queue, which is otherwise empty
                        # (p1 copies run on DVE): on sync they starve the zbuf
                        # writes -> st-slot WAR backlog -> PE gap -> HAM
                        # demotion; on a busy ACT queue they'd stall the exps.
                        # IT_ORDER puts all Q/K i-tiles first so every
                        # transpose completes before attention begins.
                        for c in range(NTR):
                            if c in tr_fired or not need_tiles[c] <= done_tiles:
                                continue
                            tr_fired.add(c)
                            nc.scalar.dma_start_transpose(
                                QKall[:, c * TRROWS:(c + 1) * TRROWS],
                                zbuf[c * TRROWS:(c + 1) * TRROWS, :])

                    # ramp set: kt-outer across 6 groups so PE work tracks the
                    # arriving k-tiles instead of waiting for all of them.
                    ramp = groups[:6]
                    rtiles = [ps1.tile([128, 512], F32, name=f"ramp{g}",
                                       tag="ps")
                              for g in range(len(ramp))]
                    for kt in range(8):
                        for g, (it, ec) in enumerate(ramp):
                            nc.tensor.matmul(
                                rtiles[g],
                                lhsT=XT[:, kt, it * 128:(it + 1) * 128],
                                rhs=WT[:, kt, ec * 512:(ec + 1) * 512],
                                start=(kt == 0), stop=(kt == 7),
                                skip_group_check=True)
                    for g, (it, ec) in enumerate(ramp):
                        emit_group(it, ec, rtiles[g])
                    done_tiles.add(IT_ORDER[0])
                    fire_transposes()

                    # steady state: kt-inner per group
                    for gi in range(6, len(groups)):
                        it, ec = groups[gi]
                        ps = ps1.tile([128, 512], F32, tag="ps")
                        for kt in range(8):
                            nc.tensor.matmul(
                                ps,
                                lhsT=XT[:, kt, it * 128:(it + 1) * 128],
                                rhs=WT[:, kt, ec * 512:(ec + 1) * 512],
                                start=(kt == 0), stop=(kt == 7))
                        emit_group(it, ec, ps)
                        if ec == 5:
                            done_tiles.add(it)
                            fire_transposes()
                            if it == 5:
                                # V rows for heads 0-3 live in i-tile 5; load
                                # them now so attention's first PV never waits.
                                # SWDGE queue: keeps sync clear for zbuf writes.
                                for h0 in range(3):
                                    nc.gpsimd.dma_start(
                                        out=vhs[h0][:, :, 0:HD],
                                        in_=zbuf[32768 + h0 * N:
                                                 32768 + (h0 + 1) * N,
                                                 0:HD].rearrange(
                                                     "(t p) d -> p t d", p=128))

            def qt_sl(h, lo, sz):
                return QKall[0:64, h * N + lo: h * N + lo + sz]

            def kt_sl(h, lo, sz):
                return QKall[0:64, 16384 + h * N + lo: 16384 + h * N + lo + sz]

            # ---------- phase 2: attention, software-pipelined ----
            # The scores+exp "produce" stream runs one (h, jt) step ahead of
            # the PV "consume" stream, across head boundaries. The window is
            # paced by ACT's exp throughput.
            with tc.tile_pool(name="pt", bufs=6) as ptpool, \
                 tc.tile_pool(name="rz", bufs=2) as rzpool, \
                 tc.tile_pool(name="mid", bufs=2) as midpool, \
                 tc.tile_pool(name="sps", bufs=2, space="PSUM") as spsum, \
                 tc.tile_pool(name="ops", bufs=2, space="PSUM") as opsum:
                steps = [(h, jt) for h in range(H) for jt in range(8)]

                def load_v(h):
                    nc.gpsimd.dma_start(
                        out=vhs[h % 4][:, :, 0:HD],
                        in_=zbuf[32768 + h * N: 32768 + (h + 1) * N,
                                 0:HD].rearrange("(t p) d -> p t d", p=128))

                def produce(h, jt):
                    sps = spsum.tile([128, 2, 512], F32, tag="sps")
                    if WARM_N:
                        # HAM warm-keeper: fills the PE's exp-wait slack so the
                        # activity monitor holds K=8/8. Overwritten by the real
                        # scores matmul (start=True).
                        nc.tensor.matmul(
                            sps[0:128, 0, 0:WARM_N],
                            lhsT=QKall[:, 0:128], rhs=QKall[:, 0:WARM_N],
                            start=True, stop=True)
                    for ic in range(2):
                        nc.tensor.matmul(
                            sps[:, ic, :],
                            lhsT=kt_sl(h, jt * 128, 128),
                            rhs=qt_sl(h, ic * 512, 512),
                            start=True, stop=True)
                    pt = ptpool.tile([128, 2, 512], BF, tag="pt")
                    if exp_ops and jt in OFFLOAD_JT:
                        # exp on DVE: (P4(s/128))^16 via two custom ops,
                        # relieving the ACT engine (the attention pacer).
                        mid = midpool.tile([128, 1024], F32, tag="mid")
                        nc.vector._custom_dve(
                            exp_ops["p4"], out=mid,
                            in0=sps.rearrange("p a b -> p (a b)"),
                            in1=c4tile, s0=EXP_C1, s1=EXP_C2, imm2=EXP_C3)
                        nc.vector._custom_dve(
                            exp_ops["p16"],
                            out=pt.rearrange("p a b -> p (a b)"), in0=mid)
                    elif EXP2D:
                        nc.scalar.activation(
                            pt.rearrange("p a b -> p (a b)"),
                            sps.rearrange("p a b -> p (a b)"),
                            FT.Exp, scale=SCALE)
                    else:
                        nc.scalar.activation(pt, sps, FT.Exp, scale=SCALE)
                    return pt

                # HAM promotion burst: ~4.5us of dependency-free back-to-back
                # matmuls. The p1 tail's stalls demote the PE clock to K=4/8;
                # promotion needs ~4us of continuous PE busy, which attention's
                # steady state never provides. This burst re-promotes once;
                # the per-step warm-keepers then hold K=8/8.
                burst = spsum.tile([128, 2, 512], F32, tag="sps")
                for _ in range(22):
                    nc.tensor.matmul(
                        burst[:, 0, :],
                        lhsT=QKall[:, 0:128], rhs=QKall[:, 0:512],
                        start=True, stop=True)

                ops = None
                deferred = []

                def consume(ops_t, pt_t, h, jt):
                    for ic in range(2):
                        nc.tensor.matmul(
                            ops_t[:, ic, :],
                            lhsT=vhs[h % 4][:, jt, :],
                            rhs=pt_t[:, ic, :],
                            start=(jt == 0), stop=(jt == 7),
                            skip_group_check=True)

                pt_next = produce(*steps[0])
                for s, (h, jt) in enumerate(steps):
                    po, hf = 64 * (h % 2), h // 2
                    if jt == 0:
                        ops = opsum.tile([128, 2, 512], F32, tag="ops")
                    ops_cur = ops
                    pt_cur = pt_next
                    if s + 1 < len(steps):
                        pt_next = produce(*steps[s + 1])
                    # DVE-offloaded steps' PV is deferred 2 steps so the PE's
                    # in-order queue never waits on the slower DVE exp chain.
                    deferred = [(d - 1, o, p, hh, jj)
                                for d, o, p, hh, jj in deferred]
                    while deferred and (deferred[0][0] <= 0 or jt == 7):
                        _, o, p, hh, jj = deferred.pop(0)
                        consume(o, p, hh, jj)
                    if exp_ops and jt in OFFLOAD_JT:
                        deferred.append((2, ops_cur, pt_cur, h, jt))
                    else:
                        consume(ops_cur, pt_cur, h, jt)
                    if jt == 5 and h + 3 < H:
                        load_v(h + 3)
                    if jt == 7:
                        # custom-DVE reciprocal can't read PSUM; stage the
                        # denominator through SBUF. One fused chain per head.
                        zst = rzpool.tile([64, 2, 512], F32, tag="zst")
                        nc.vector.tensor_copy(zst, ops_cur[64:128, :, :])
                        rzs = rzpool.tile([64, 2, 512], F32, tag="rzs")
                        nc.vector.reciprocal_approx_fast(rzs, zst)
                        nc.vector.tensor_mul(
                            oT[po:po + 64, hf, :, :],
                            ops_cur[0:64, :, :], rzs)

            # ---------- phase 3: out = oT.T @ w_out^T + b ----------
            with tc.tile_pool(name="p3st", bufs=4) as p3st, \
                 tc.tile_pool(name="ps3", bufs=4, space="PSUM") as ps3:
                for it in range(8):
                    for ec in range(2):
                        rps = ps3.tile([128, 512], F32)
                        for ct in range(8):
                            nc.tensor.matmul(
                                rps,
                                lhsT=oT[:, ct, it // 4,
                                        (it % 4) * 128:(it % 4 + 1) * 128],
                                rhs=WOT[:, ct, ec * 512:(ec + 1) * 512],
                                start=(ct == 0), stop=(ct == 7))
                        ost = p3st.tile([128, 512], F32)
                        nc.vector.tensor_add(
                            ost, rps, biasrep[:, ec * 512:(ec + 1) * 512])
                        nc.sync.dma_start(
                            out=outp[it * 128:(it + 1) * 128,
                                     ec * 512:(ec + 1) * 512],
                            in_=ost)

    nc.finalize()
    return nc


_CACHE = {}


def _get_nc():
    if "nc" not in _CACHE:
        _CACHE["nc"] = build()
    return _CACHE["nc"]


def make_in_maps(x, w_qkv, w_out, b_out):
    bf = ml_dtypes.bfloat16
    wqkvt = np.ascontiguousarray(np.asarray(w_qkv, dtype=np.float32).T).astype(bf)
    woutt = np.ascontiguousarray(np.asarray(w_out, dtype=np.float32).T).astype(bf)
    bias = np.ascontiguousarray(np.asarray(b_out, dtype=np.float32).reshape(1, DIM))
    x = np.asarray(x, dtype=np.float32)
    return [
        {
            "xt": np.ascontiguousarray(x[b].T).astype(bf),
            "wqkvt": wqkvt,
            "woutt": woutt,
            "bias": bias,
        }
        for b in range(N_CORES)
    ]


def kernel(x, w_qkv, w_out, b_out):
    nc = _get_nc()
    in_maps = make_in_maps(x, w_qkv, w_out, b_out)
    res = run_bass_kernel_spmd(nc, in_maps, core_ids=list(range(N_CORES)))
    return np.stack(
        [res.results[b]["out"] for b in range(N_CORES)], axis=0
    ).astype(np.float32)


# revision 44
# speedup vs baseline: 1.2866x; 1.0979x over previous
"""Trainium2 Bass kernel for nn_Attention_2851858284976.

Dense transformer attention block, b=8 n=1024 dim=1024 heads=16.
Sharding: pure data parallel - one batch element per NeuronCore (8 cores).

Per-core math (batch element x of shape (n, dim)):
  Y = x @ w_qkv^T                              (n, 3*dim)
  Z = Y.reshape(49152, 64)   # raw reshape: rows are (token, col-block) pairs
  Q = Z[0:16384], K = Z[16384:32768], V = Z[32768:49152], each (16, 1024, 64)
  per head: P^T = exp(scale * K_h @ Q_h^T)     (softmax along the partition axis)
            [O^T; Zs*64] = [V_h | 1*64]^T @ P^T  (ones cols replicate the denom)
            oT_h = O^T * (1/Zs)
  out = (oT stacked).T @ w_out^T + b_out

Structure vs the previous version:
  - Phase 1 runs kt-OUTER over a ramp set of 8 PSUM banks so the first
    matmuls issue as soon as the first k-tile of xt/wqkvt lands (instead of
    waiting ~30us for the full 8MB input DMA).
  - DMA xbar transposes are issued from the sync queue (not ACT's queue).
  - No HAM warm-keeper matmuls in the attention loop.
  - exp reads/writes 2D contiguous APs (3D APs pay a per-outer-dim init).
  - The softmax denominator chain is one fused (64,2,512) copy/recip/mul
    per head instead of two per-ic chains.
  - exp activation table preloaded during phase 1.
"""
import numpy as np
import ml_dtypes

import concourse.bass as bass
import concourse.mybir as mybir
from concourse import bacc
from concourse.tile import TileContext
from concourse.bass_utils import run_bass_kernel_spmd

N_CORES = 8
N = 1024          # tokens
DIM = 1024
E3 = 3 * DIM      # qkv projection width
H = 16            # heads
HD = 64           # head dim
SCALE = HD ** -0.5
ZROWS = N * E3 // HD          # 49152 Z-rows, padded to 128 wide in DRAM

F32 = mybir.dt.float32
BF = mybir.dt.bfloat16
FT = mybir.ActivationFunctionType

# i-tile emission order: gets Q/K/V Z-row producers done in the order the
# attention heads consume them (Q:c0<-it0, K first chunks<-it2/3, V h0-3<-it5).
IT_ORDER = [0, 2, 3, 1, 4, 5, 6, 7]

EXP2D = True      # exp via 2D contiguous APs
WARM_N = 0        # HAM warm-keeper matmul width (0 = off)
OFFLOAD_JT = ()   # jt steps whose exp runs on DVE instead of ACT (net loss:
                  # DVE FIFO couples the exp chain with the denominator
                  # chains and the sps lifetime extension stalls scores)

# exp(s*SCALE) = (P4(s/128))^16 with P4 = deg-4 Taylor of e^u at u=s/128.
# |s| <= ~45 on this data -> |u| <= 0.35 -> approx rel err < 8e-4.
EXP_A = SCALE / 16.0
EXP_C1 = EXP_A
EXP_C2 = EXP_A * EXP_A / 2.0
EXP_C3 = EXP_A ** 3 / 6.0
EXP_C4 = EXP_A ** 4 / 24.0

_EXP_OPS = {}


def _register_exp_ops():
    """Register two custom DVE ops (poly-4 eval, ^16) used to offload part of
    the softmax exp from ACT to the otherwise-idle vector engine."""
    if _EXP_OPS:
        return _EXP_OPS
    import concourse.dve_ops as dve_ops
    from concourse.dve_spec import (
        Spec, Src0, C0, C1, C2, C3, One, lower, sq, _spill_c3_to_src1,
        _has_src1)
    from concourse.dve_uop import DveOpSpec

    def reg(name, spec):
        for op in dve_ops.OPS:
            if op.name == name:
                return op
        row = dve_ops._CUSTOM_DVE_ROW_BASE + len(dve_ops.OPS)
        shas = {}
        for ver in ("v3", "v4"):
            s = DveOpSpec(name=name, opcode=row, uops=lower(spec, ver=ver),
                          rd1_en=_has_src1(spec))
            shas[ver] = s.sha(ver)
        op = dve_ops.DveOp(name, spec, subdim=False, uops_sha=shas)
        dve_ops.OPS.append(op)
        dve_ops.CUSTOM_DVE_SPECS[name] = spec
        dve_ops._SUB_OPCODE_FOR_NAME[name] = row
        return op

    body = ((((C3 * Src0 + C2) * Src0 + C1) * Src0 + C0) * Src0 + One)
    p4 = reg("ANT_EXP_P4", Spec(
        body=_spill_c3_to_src1(body),
        reference=lambda in0, in1, s0, s1, imm2:
            ((((in1 * in0 + imm2) * in0 + s1) * in0 + s0) * in0 + 1.0)))
    p16 = reg("ANT_POW16", Spec(
        body=sq(sq(sq(sq(Src0)))),
        reference=lambda in0, in1, s0, s1, imm2: in0 ** 16))
    _EXP_OPS["p4"] = p4
    _EXP_OPS["p16"] = p16
    return _EXP_OPS


def build():
    nc = bacc.Bacc("TRN2", target_bir_lowering=False, num_devices=N_CORES)
    xt = nc.declare_dram_parameter("xt", [DIM, N], BF, isOutput=False)
    wqkvt = nc.declare_dram_parameter("wqkvt", [DIM, E3], BF, isOutput=False)
    woutt = nc.declare_dram_parameter("woutt", [DIM, DIM], BF, isOutput=False)
    bias = nc.declare_dram_parameter("bias", [1, DIM], F32, isOutput=False)
    outp = nc.declare_dram_parameter("out", [N, DIM], F32, isOutput=True)

    with TileContext(nc) as tc:
        with tc.tile_pool(name="dram", bufs=1, space="DRAM") as dpool, \
             tc.tile_pool(name="singles", bufs=1) as singles:
            zbuf = dpool.tile([ZROWS, 128], BF)    # Z rows, cols 64:128 unused
            zb3 = zbuf.rearrange("(r c) d -> r c d", c=48)   # (1024, 48, 128)

            # oT indexed [64*(h%2)+dd, h//2, ic, i] so the per-head denominator
            # mul and the phase-3 lhsT slices are both natural APs.
            oT = singles.tile([128, 8, 2, 512], BF)
            biasrep = singles.tile([128, DIM], F32)
            QKall = singles.tile([128, 32768], BF)
            WOT = singles.tile([128, 8, DIM], BF)

            ones_f = singles.tile([128, 8, HD], F32)
            nc.vector.memset(ones_f, 1.0)
            # [V | ones*64] stationary tiles for the PV matmul; ones half gives
            # the softmax denominator replicated on out rows 64-127.
            vh0 = singles.tile([128, 8, 2 * HD], BF)
            vh1 = singles.tile([128, 8, 2 * HD], BF)
            vh2 = singles.tile([128, 8, 2 * HD], BF)
            vh3 = singles.tile([128, 8, 2 * HD], BF)
            vhs = [vh0, vh1, vh2, vh3]
            for v in vhs:
                nc.vector.tensor_copy(v[:, :, HD:2 * HD], ones_f)

            # Preload the exp activation table while the PE ramps (saves the
            # ~2.7us table-load landing inside the attention window).
            expwarm = singles.tile([1, 8], F32)
            nc.vector.memset(expwarm, 0.0)
            nc.scalar.activation(expwarm, expwarm, FT.Exp, scale=1.0)

            exp_ops = _register_exp_ops() if OFFLOAD_JT else None
            if exp_ops:
                c4tile = singles.tile([128, 1], F32)
                nc.vector.memset(c4tile, EXP_C4)

            # ---------- phase 1: Y = x @ w_qkv^T -> zbuf ----------
            with tc.tile_pool(name="p1io", bufs=1) as p1io:
                XT = p1io.tile([128, 8, N], BF)
                WT = p1io.tile([128, 8, E3], BF)
                # kt-major input loads so the ramp set's kt=0 matmuls can
                # start after ~1MB of DMA instead of 8MB.
                for kt in range(8):
                    nc.sync.dma_start(
                        out=XT[:, kt, :], in_=xt[kt * 128:(kt + 1) * 128, :])
                    nc.sync.dma_start(
                        out=WT[:, kt, :], in_=wqkvt[kt * 128:(kt + 1) * 128, :])
                nc.sync.dma_start(
                    out=WOT, in_=woutt[:].rearrange("(a p) e -> p a e", p=128))
                nc.sync.dma_start(
                    out=biasrep, in_=bias[:].to_broadcast((128, DIM)))

                groups = [(it, ec) for it in IT_ORDER for ec in range(6)]
                # transpose chunk c covers Z-rows [4096c, 4096(c+1)) = Y rows
                # [85.33c, 85.33(c+1)); fire it once the covering i-tiles have
                # written zbuf. (16 finer chunks measured slower: per-chunk
                # descriptor-gen overhead outweighs the smaller DRAM bursts.)
                NTR = 8
                TRROWS = ZROWS * 2 // 3 // NTR
                need_tiles = {
                    c: set(range((TRROWS * c) // 48 // 128,
                                 ((TRROWS * (c + 1) - 1) // 48) // 128 + 1))
                    for c in range(NTR)
                }
                done_tiles = set()
                tr_fired = set()

                with tc.tile_pool(name="p1st", bufs=8) as p1st, \
                     tc.tile_pool(name="ps1", bufs=6, space="PSUM") as ps1:

                    def emit_group(it, ec, ps):
                        # copies on DVE: keeps ACT's queue empty so the first
                        # attention exp isn't stuck behind a copy backlog, and
                        # leaves 2 PSUM banks untouched by p1 so the attention
                        # pools allocate without waiting on the copy tail.
                        st = p1st.tile([128, 512], BF)
                        nc.vector.tensor_copy(st, ps)
                        nc.sync.dma_start(
                            out=zb3[it * 128:(it + 1) * 128,
                                    ec * 8:(ec + 1) * 8, 0:HD],
                            in_=st.rearrange("p (b d) -> p b d", d=HD))

                    def fire_transposes():
                        # transposes on ACT's queue, which is otherwise empty
                        # (p1 copies run on DVE): on sync they starve the zbuf
                        # writes -> st-slot WAR backlog -> PE gap -> HAM
                        # demotion; on a busy ACT queue they'd stall the exps.
                        # IT_ORDER puts all Q/K i-tiles first so every
                        # transpose completes before attention begins.
                        for c in range(NTR):
                            if c in tr_fired or not need_tiles[c] <= done_tiles:
                                continue
                            tr_fired.add(c)
                            nc.scalar.dma_start_transpose(
                                QKall[:, c * TRROWS:(c + 1) * TRROWS],
                                zbuf[c * TRROWS:(c + 1) * TRROWS, :])

                    # ramp set: kt-outer across 6 groups so PE work tracks the
                    # arriving k-tiles instead of waiting for all of them.
                    ramp = groups[:6]
                    rtiles = [ps1.tile([128, 512], F32, name=f"ramp{g}",
                                       tag="ps")
                              for g in range(len(ramp))]
                    for kt in range(8):
                        for g, (it, ec) in enumerate(ramp):
                            nc.tensor.matmul(
                                rtiles[g],
                                lhsT=XT[:, kt, it * 128:(it + 1) * 128],
                                rhs=WT[:, kt, ec * 512:(ec + 1) * 512],
                                start=(kt == 0), stop=(kt == 7),
                                skip_group_check=True)
                    for g, (it, ec) in enumerate(ramp):
                        emit_group(it, ec, rtiles[g])
                    done_tiles.add(IT_ORDER[0])
                    fire_transposes()

                    # steady state: kt-inner per group
                    for gi in range(6, len(groups)):
                        it, ec = groups[gi]
                        ps = ps1.tile([128, 512], F32, tag="ps")
                        for kt in range(8):
                            nc.tensor.matmul(
                                ps,
                                lhsT=XT[:, kt, it * 128:(it + 1) * 128],
                                rhs=WT[:, kt, ec * 512:(ec + 1) * 512],
                                start=(kt == 0), stop=(kt == 7))
                        emit_group(it, ec, ps)
                        if ec == 5:
                            done_tiles.add(it)
                            fire_transposes()
                            if it == 5:
                                # V rows for heads 0-3 live in i-tile 5; load
                                # them now so attention's first PV never waits.
                                # SWDGE queue: keeps sync clear for zbuf writes.
                                for h0 in range(3):
                                    nc.gpsimd.dma_start(
                                        out=vhs[h0][:, :, 0:HD],
                                        in_=zbuf[32768 + h0 * N:
                                                 32768 + (h0 + 1) * N,
                                                 0:HD].rearrange(
                                                     "(t p) d -> p t d", p=128))

            def qt_sl(h, lo, sz):
                return QKall[0:64, h * N + lo: h * N + lo + sz]

            def kt_sl(h, lo, sz):
                return QKall[0:64, 16384 + h * N + lo: 16384 + h * N + lo + sz]

            # ---------- phase 2: attention, software-pipelined ----
            # The scores+exp "produce" stream runs one (h, jt) step ahead of
            # the PV "consume" stream, across head boundaries. The window is
            # paced by ACT's exp throughput.
            with tc.tile_pool(name="pt", bufs=6) as ptpool, \
                 tc.tile_pool(name="rz", bufs=2) as rzpool, \
                 tc.tile_pool(name="mid", bufs=2) as midpool, \
                 tc.tile_pool(name="sps", bufs=2, space="PSUM") as spsum, \
                 tc.tile_pool(name="ops", bufs=2, space="PSUM") as opsum:
                steps = [(h, jt) for h in range(H) for jt in range(8)]

                def load_v(h):
                    nc.gpsimd.dma_start(
                        out=vhs[h % 4][:, :, 0:HD],
                        in_=zbuf[32768 + h * N: 32768 + (h + 1) * N,
                                 0:HD].rearrange("(t p) d -> p t d", p=128))

                def produce(h, jt):
                    sps = spsum.tile([128, 2, 512], F32, tag="sps")
                    if WARM_N:
                        # HAM warm-keeper: fills the PE's exp-wait slack so the
                        # activity monitor holds K=8/8. Overwritten by the real
                        # scores matmul (start=True).
                        nc.tensor.matmul(
                            sps[0:128, 0, 0:WARM_N],
                            lhsT=QKall[:, 0:128], rhs=QKall[:, 0:WARM_N],
                            start=True, stop=True)
                    for ic in range(2):
                        nc.tensor.matmul(
                            sps[:, ic, :],
                            lhsT=kt_sl(h, jt * 128, 128),
                            rhs=qt_sl(h, ic * 512, 512),
                            start=True, stop=True)
                    pt = ptpool.tile([128, 2, 512], BF, tag="pt")
                    if exp_ops and jt in OFFLOAD_JT:
                        # exp on DVE: (P4(s/128))^16 via two custom ops,
                        # relieving the ACT engine (the attention pacer).
                        mid = midpool.tile([128, 1024], F32, tag="mid")
                        nc.vector._custom_dve(
                            exp_ops["p4"], out=mid,
                            in0=sps.rearrange("p a b -> p (a b)"),
                            in1=c4tile, s0=EXP_C1, s1=EXP_C2, imm2=EXP_C3)
                        nc.vector._custom_dve(
                            exp_ops["p16"],
                            out=pt.rearrange("p a b -> p (a b)"), in0=mid)
                    elif EXP2D:
                        nc.scalar.activation(
                            pt.rearrange("p a b -> p (a b)"),
                            sps.rearrange("p a b -> p (a b)"),
                            FT.Exp, scale=SCALE)
                    else:
                        nc.scalar.activation(pt, sps, FT.Exp, scale=SCALE)
                    return pt

                # HAM promotion burst: ~4.5us of dependency-free back-to-back
                # matmuls. The p1 tail's stalls demote the PE clock to K=4/8;
                # promotion needs ~4us of continuous PE busy, which attention's
                # steady state never provides. This burst re-promotes once;
                # the per-step warm-keepers then hold K=8/8.
                burst = spsum.tile([128, 2, 512], F32, tag="sps")
                for _ in range(22):
                    nc.tensor.matmul(
                        burst[:, 0, :],
                        lhsT=QKall[:, 0:128], rhs=QKall[:, 0:512],
                        start=True, stop=True)

                ops = None
                deferred = []

                def consume(ops_t, pt_t, h, jt):
                    for ic in range(2):
                        nc.tensor.matmul(
                            ops_t[:, ic, :],
                            lhsT=vhs[h % 4][:, jt, :],
                            rhs=pt_t[:, ic, :],
                            start=(jt == 0), stop=(jt == 7),
                            skip_group_check=True)

                pt_next = produce(*steps[0])
                for s, (h, jt) in enumerate(steps):
                    po, hf = 64 * (h % 2), h // 2
                    if jt == 0:
                        ops = opsum.tile([128, 2, 512], F32, tag="ops")
                    ops_cur = ops
                    pt_cur = pt_next
                    if s + 1 < len(steps):
                        pt_next = produce(*steps[s + 1])
                    # DVE-offloaded steps' PV is deferred 2 steps so the PE's
                    # in-order queue never waits on the slower DVE exp chain.
                    deferred = [(d - 1, o, p, hh, jj)
                                for d, o, p, hh, jj in deferred]
                    while deferred and (deferred[0][0] <= 0 or jt == 7):
                        _, o, p, hh, jj = deferred.pop(0)
                        consume(o, p, hh, jj)
                    if exp_ops and jt in OFFLOAD_JT:
                        deferred.append((2, ops_cur, pt_cur, h, jt))
                    else:
                        consume(ops_cur, pt_cur, h, jt)
                    if jt == 5 and h + 3 < H:
                        load_v(h + 3)
                    if jt == 7:
                        # custom-DVE reciprocal can't read PSUM; stage the
                        # denominator through SBUF. One fused chain per head.
                        zst = rzpool.tile([64, 2, 512], F32, tag="zst")
                        nc.vector.tensor_copy(zst, ops_cur[64:128, :, :])
                        rzs = rzpool.tile([64, 2, 512], F32, tag="rzs")
                        nc.vector.reciprocal_approx_fast(rzs, zst)
                        nc.vector.tensor_mul(
                            oT[po:po + 64, hf, :, :],
                            ops_cur[0:64, :, :], rzs)

            # ---------- phase 3: out = oT.T @ w_out^T + b ----------
            with tc.tile_pool(name="p3st", bufs=4) as p3st, \
                 tc.tile_pool(name="ps3", bufs=4, space="PSUM") as ps3:
                for it in range(8):
                    for ec in range(2):
                        rps = ps3.tile([128, 512], F32)
                        for ct in range(8):
                            nc.tensor.matmul(
                                rps,
                                lhsT=oT[:, ct, it // 4,
                                        (it % 4) * 128:(it % 4 + 1) * 128],
                                rhs=WOT[:, ct, ec * 512:(ec + 1) * 512],
                                start=(ct == 0), stop=(ct == 7))
                        ost = p3st.tile([128, 512], F32)
                        nc.vector.tensor_add(
                            ost, rps, biasrep[:, ec * 512:(ec + 1) * 512])
                        nc.sync.dma_start(
                            out=outp[it * 128:(it + 1) * 128,
                                     ec * 512:(ec + 1) * 512],
                            in_=ost)

    nc.finalize()
    return nc


_CACHE = {}


def _get_nc():
    if "nc" not in _CACHE:
        _CACHE["nc"] = build()
    return _CACHE["nc"]


def make_in_maps(x, w_qkv, w_out, b_out):
    bf = ml_dtypes.bfloat16
    wqkvt = np.ascontiguousarray(np.asarray(w_qkv, dtype=np.float32).T).astype(bf)
    woutt = np.ascontiguousarray(np.asarray(w_out, dtype=np.float32).T).astype(bf)
    bias = np.ascontiguousarray(np.asarray(b_out, dtype=np.float32).reshape(1, DIM))
    x = np.asarray(x, dtype=np.float32)
    return [
        {
            "xt": np.ascontiguousarray(x[b].T).astype(bf),
            "wqkvt": wqkvt,
            "woutt": woutt,
            "bias": bias,
        }
        for b in range(N_CORES)
    ]


def kernel(x, w_qkv, w_out, b_out):
    nc = _get_nc()
    in_maps = make_in_maps(x, w_qkv, w_out, b_out)
    res = run_bass_kernel_spmd(nc, in_maps, core_ids=list(range(N_CORES)))
    return np.stack(
        [res.results[b]["out"] for b in range(N_CORES)], axis=0
    ).astype(np.float32)


# revision 48
# speedup vs baseline: 1.2927x; 1.0048x over previous
"""Trainium2 Bass kernel for nn_Attention_2851858284976.

Dense transformer attention block, b=8 n=1024 dim=1024 heads=16.
Sharding: pure data parallel - one batch element per NeuronCore (8 cores).

Per-core math (batch element x of shape (n, dim)):
  Y = x @ w_qkv^T                              (n, 3*dim)
  Z = Y.reshape(49152, 64)   # raw reshape: rows are (token, col-block) pairs
  Q = Z[0:16384], K = Z[16384:32768], V = Z[32768:49152], each (16, 1024, 64)
  per head: P^T = exp(scale * K_h @ Q_h^T)     (softmax along the partition axis)
            [O^T; Zs*64] = [V_h | 1*64]^T @ P^T  (ones cols replicate the denom)
            oT_h = O^T * (1/Zs)
  out = (oT stacked).T @ w_out^T + b_out

Structure vs the previous version:
  - Phase 1 runs kt-OUTER over a ramp set of 8 PSUM banks so the first
    matmuls issue as soon as the first k-tile of xt/wqkvt lands (instead of
    waiting ~30us for the full 8MB input DMA).
  - DMA xbar transposes are issued from the sync queue (not ACT's queue).
  - No HAM warm-keeper matmuls in the attention loop.
  - exp reads/writes 2D contiguous APs (3D APs pay a per-outer-dim init).
  - The softmax denominator chain is one fused (64,2,512) copy/recip/mul
    per head instead of two per-ic chains.
  - exp activation table preloaded during phase 1.
"""
import numpy as np
import ml_dtypes

import concourse.bass as bass
import concourse.mybir as mybir
from concourse import bacc
from concourse.tile import TileContext
from concourse.bass_utils import run_bass_kernel_spmd

N_CORES = 8
N = 1024          # tokens
DIM = 1024
E3 = 3 * DIM      # qkv projection width
H = 16            # heads
HD = 64           # head dim
SCALE = HD ** -0.5
ZROWS = N * E3 // HD          # 49152 Z-rows, padded to 128 wide in DRAM

F32 = mybir.dt.float32
BF = mybir.dt.bfloat16
FT = mybir.ActivationFunctionType

# i-tile emission order: gets Q/K/V Z-row producers done in the order the
# attention heads consume them (Q:c0<-it0, K first chunks<-it2/3, V h0-3<-it5).
IT_ORDER = [0, 2, 3, 1, 4, 5, 6, 7]

EXP2D = True      # exp via 2D contiguous APs
WARM_N = 0        # HAM warm-keeper matmul width (0 = off)
OFFLOAD_JT = ()   # jt steps whose exp runs on DVE instead of ACT (net loss:
                  # DVE FIFO couples the exp chain with the denominator
                  # chains and the sps lifetime extension stalls scores)

# exp(s*SCALE) = (P4(s/128))^16 with P4 = deg-4 Taylor of e^u at u=s/128.
# |s| <= ~45 on this data -> |u| <= 0.35 -> approx rel err < 8e-4.
EXP_A = SCALE / 16.0
EXP_C1 = EXP_A
EXP_C2 = EXP_A * EXP_A / 2.0
EXP_C3 = EXP_A ** 3 / 6.0
EXP_C4 = EXP_A ** 4 / 24.0

_EXP_OPS = {}


def _register_exp_ops():
    """Register two custom DVE ops (poly-4 eval, ^16) used to offload part of
    the softmax exp from ACT to the otherwise-idle vector engine."""
    if _EXP_OPS:
        return _EXP_OPS
    import concourse.dve_ops as dve_ops
    from concourse.dve_spec import (
        Spec, Src0, C0, C1, C2, C3, One, lower, sq, _spill_c3_to_src1,
        _has_src1)
    from concourse.dve_uop import DveOpSpec

    def reg(name, spec):
        for op in dve_ops.OPS:
            if op.name == name:
                return op
        row = dve_ops._CUSTOM_DVE_ROW_BASE + len(dve_ops.OPS)
        shas = {}
        for ver in ("v3", "v4"):
            s = DveOpSpec(name=name, opcode=row, uops=lower(spec, ver=ver),
                          rd1_en=_has_src1(spec))
            shas[ver] = s.sha(ver)
        op = dve_ops.DveOp(name, spec, subdim=False, uops_sha=shas)
        dve_ops.OPS.append(op)
        dve_ops.CUSTOM_DVE_SPECS[name] = spec
        dve_ops._SUB_OPCODE_FOR_NAME[name] = row
        return op

    body = ((((C3 * Src0 + C2) * Src0 + C1) * Src0 + C0) * Src0 + One)
    p4 = reg("ANT_EXP_P4", Spec(
        body=_spill_c3_to_src1(body),
        reference=lambda in0, in1, s0, s1, imm2:
            ((((in1 * in0 + imm2) * in0 + s1) * in0 + s0) * in0 + 1.0)))
    p16 = reg("ANT_POW16", Spec(
        body=sq(sq(sq(sq(Src0)))),
        reference=lambda in0, in1, s0, s1, imm2: in0 ** 16))
    _EXP_OPS["p4"] = p4
    _EXP_OPS["p16"] = p16
    return _EXP_OPS


def build():
    nc = bacc.Bacc("TRN2", target_bir_lowering=False, num_devices=N_CORES)
    xt = nc.declare_dram_parameter("xt", [DIM, N], BF, isOutput=False)
    wqkvt = nc.declare_dram_parameter("wqkvt", [DIM, E3], BF, isOutput=False)
    woutt = nc.declare_dram_parameter("woutt", [DIM, DIM], BF, isOutput=False)
    bias = nc.declare_dram_parameter("bias", [1, DIM], F32, isOutput=False)
    outp = nc.declare_dram_parameter("out", [N, DIM], F32, isOutput=True)

    with TileContext(nc) as tc:
        with tc.tile_pool(name="dram", bufs=1, space="DRAM") as dpool, \
             tc.tile_pool(name="singles", bufs=1) as singles:
            zbuf = dpool.tile([ZROWS, 128], BF)    # Z rows, cols 64:128 unused
            zb3 = zbuf.rearrange("(r c) d -> r c d", c=48)   # (1024, 48, 128)

            # oT indexed [64*(h%2)+dd, h//2, ic, i] so the per-head denominator
            # mul and the phase-3 lhsT slices are both natural APs.
            oT = singles.tile([128, 8, 2, 512], BF)
            biasrep = singles.tile([128, DIM], F32)
            QKall = singles.tile([128, 32768], BF)
            WOT = singles.tile([128, 8, DIM], BF)

            ones_f = singles.tile([128, 8, HD], F32)
            nc.vector.memset(ones_f, 1.0)
            # [V | ones*64] stationary tiles for the PV matmul; ones half gives
            # the softmax denominator replicated on out rows 64-127.
            vh0 = singles.tile([128, 8, 2 * HD], BF)
            vh1 = singles.tile([128, 8, 2 * HD], BF)
            vh2 = singles.tile([128, 8, 2 * HD], BF)
            vh3 = singles.tile([128, 8, 2 * HD], BF)
            vhs = [vh0, vh1, vh2, vh3]
            for v in vhs:
                nc.vector.tensor_copy(v[:, :, HD:2 * HD], ones_f)

            # Preload the exp activation table while the PE ramps (saves the
            # ~2.7us table-load landing inside the attention window).
            expwarm = singles.tile([1, 8], F32)
            nc.vector.memset(expwarm, 0.0)
            nc.scalar.activation(expwarm, expwarm, FT.Exp, scale=1.0)

            exp_ops = _register_exp_ops() if OFFLOAD_JT else None
            if exp_ops:
                c4tile = singles.tile([128, 1], F32)
                nc.vector.memset(c4tile, EXP_C4)

            # ---------- phase 1: Y = x @ w_qkv^T -> zbuf ----------
            with tc.tile_pool(name="p1io", bufs=1) as p1io:
                XT = p1io.tile([128, 8, N], BF)
                WT = p1io.tile([128, 8, E3], BF)
                # kt-major input loads so the ramp set's kt=0 matmuls can
                # start after ~1MB of DMA instead of 8MB.
                for kt in range(8):
                    nc.sync.dma_start(
                        out=XT[:, kt, :], in_=xt[kt * 128:(kt + 1) * 128, :])
                    nc.sync.dma_start(
                        out=WT[:, kt, :], in_=wqkvt[kt * 128:(kt + 1) * 128, :])
                nc.sync.dma_start(
                    out=WOT, in_=woutt[:].rearrange("(a p) e -> p a e", p=128))
                nc.sync.dma_start(
                    out=biasrep, in_=bias[:].to_broadcast((128, DIM)))

                groups = [(it, ec) for it in IT_ORDER for ec in range(6)]
                # transpose chunk c covers Z-rows [4096c, 4096(c+1)) = Y rows
                # [85.33c, 85.33(c+1)); fire it once the covering i-tiles have
                # written zbuf. (16 finer chunks measured slower: per-chunk
                # descriptor-gen overhead outweighs the smaller DRAM bursts.)
                NTR = 8
                TRROWS = ZROWS * 2 // 3 // NTR
                need_tiles = {
                    c: set(range((TRROWS * c) // 48 // 128,
                                 ((TRROWS * (c + 1) - 1) // 48) // 128 + 1))
                    for c in range(NTR)
                }
                done_tiles = set()
                tr_fired = set()

                with tc.tile_pool(name="p1st", bufs=8) as p1st, \
                     tc.tile_pool(name="ps1", bufs=6, space="PSUM") as ps1:

                    def emit_group(it, ec, ps):
                        # copies on DVE: keeps ACT's queue empty so the first
                        # attention exp isn't stuck behind a copy backlog, and
                        # leaves 2 PSUM banks untouched by p1 so the attention
                        # pools allocate without waiting on the copy tail.
                        st = p1st.tile([128, 512], BF)
                        nc.vector.tensor_copy(st, ps)
                        nc.sync.dma_start(
                            out=zb3[it * 128:(it + 1) * 128,
                                    ec * 8:(ec + 1) * 8, 0:HD],
                            in_=st.rearrange("p (b d) -> p b d", d=HD))

                    def fire_transposes():
                        # transposes on ACT's queue, which is otherwise empty
                        # (p1 copies run on DVE): on sync they starve the zbuf
                        # writes -> st-slot WAR backlog -> PE gap -> HAM
                        # demotion; on a busy ACT queue they'd stall the exps.
                        # IT_ORDER puts all Q/K i-tiles first so every
                        # transpose completes before attention begins.
                        # (Deferring the late chunks into the attention window
                        # measured slower AND raced the attention reads.)
                        for c in range(NTR):
                            if c in tr_fired or not need_tiles[c] <= done_tiles:
                                continue
                            tr_fired.add(c)
                            nc.scalar.dma_start_transpose(
                                QKall[:, c * TRROWS:(c + 1) * TRROWS],
                                zbuf[c * TRROWS:(c + 1) * TRROWS, :])

                    # ramp set: kt-outer across 6 groups so PE work tracks the
                    # arriving k-tiles instead of waiting for all of them.
                    ramp = groups[:6]
                    rtiles = [ps1.tile([128, 512], F32, name=f"ramp{g}",
                                       tag="ps")
                              for g in range(len(ramp))]
                    for kt in range(8):
                        for g, (it, ec) in enumerate(ramp):
                            nc.tensor.matmul(
                                rtiles[g],
                                lhsT=XT[:, kt, it * 128:(it + 1) * 128],
                                rhs=WT[:, kt, ec * 512:(ec + 1) * 512],
                                start=(kt == 0), stop=(kt == 7),
                                skip_group_check=True)
                    for g, (it, ec) in enumerate(ramp):
                        emit_group(it, ec, rtiles[g])
                    done_tiles.add(IT_ORDER[0])
                    fire_transposes()

                    # steady state: kt-inner per group
                    for gi in range(6, len(groups)):
                        it, ec = groups[gi]
                        ps = ps1.tile([128, 512], F32, tag="ps")
                        for kt in range(8):
                            nc.tensor.matmul(
                                ps,
                                lhsT=XT[:, kt, it * 128:(it + 1) * 128],
                                rhs=WT[:, kt, ec * 512:(ec + 1) * 512],
                                start=(kt == 0), stop=(kt == 7))
                        emit_group(it, ec, ps)
                        if ec == 5:
                            done_tiles.add(it)
                            fire_transposes()
                            if it == 5:
                                # V rows for heads 0-3 live in i-tile 5; load
                                # them now so attention's first PV never waits.
                                # SWDGE queue: keeps sync clear for zbuf writes.
                                for h0 in range(3):
                                    nc.gpsimd.dma_start(
                                        out=vhs[h0][:, :, 0:HD],
                                        in_=zbuf[32768 + h0 * N:
                                                 32768 + (h0 + 1) * N,
                                                 0:HD].rearrange(
                                                     "(t p) d -> p t d", p=128))

            def qt_sl(h, lo, sz):
                return QKall[0:64, h * N + lo: h * N + lo + sz]

            def kt_sl(h, lo, sz):
                return QKall[0:64, 16384 + h * N + lo: 16384 + h * N + lo + sz]

            # ---------- phase 2: attention, software-pipelined ----
            # The scores+exp "produce" stream runs one (h, jt) step ahead of
            # the PV "consume" stream, across head boundaries. The window is
            # paced by ACT's exp throughput.
            with tc.tile_pool(name="pt", bufs=6) as ptpool, \
                 tc.tile_pool(name="rz", bufs=2) as rzpool, \
                 tc.tile_pool(name="mid", bufs=2) as midpool, \
                 tc.tile_pool(name="sps", bufs=2, space="PSUM") as spsum, \
                 tc.tile_pool(name="ops", bufs=2, space="PSUM") as opsum:
                steps = [(h, jt) for h in range(H) for jt in range(8)]

                def load_v(h):
                    nc.gpsimd.dma_start(
                        out=vhs[h % 4][:, :, 0:HD],
                        in_=zbuf[32768 + h * N: 32768 + (h + 1) * N,
                                 0:HD].rearrange("(t p) d -> p t d", p=128))

                def produce(h, jt):
                    sps = spsum.tile([128, 2, 512], F32, tag="sps")
                    if WARM_N:
                        # HAM warm-keeper: fills the PE's exp-wait slack so the
                        # activity monitor holds K=8/8. Overwritten by the real
                        # scores matmul (start=True).
                        nc.tensor.matmul(
                            sps[0:128, 0, 0:WARM_N],
                            lhsT=QKall[:, 0:128], rhs=QKall[:, 0:WARM_N],
                            start=True, stop=True)
                    for ic in range(2):
                        nc.tensor.matmul(
                            sps[:, ic, :],
                            lhsT=kt_sl(h, jt * 128, 128),
                            rhs=qt_sl(h, ic * 512, 512),
                            start=True, stop=True)
                    pt = ptpool.tile([128, 2, 512], BF, tag="pt")
                    if exp_ops and jt in OFFLOAD_JT:
                        # exp on DVE: (P4(s/128))^16 via two custom ops,
                        # relieving the ACT engine (the attention pacer).
                        mid = midpool.tile([128, 1024], F32, tag="mid")
                        nc.vector._custom_dve(
                            exp_ops["p4"], out=mid,
                            in0=sps.rearrange("p a b -> p (a b)"),
                            in1=c4tile, s0=EXP_C1, s1=EXP_C2, imm2=EXP_C3)
                        nc.vector._custom_dve(
                            exp_ops["p16"],
                            out=pt.rearrange("p a b -> p (a b)"), in0=mid)
                    elif EXP2D:
                        nc.scalar.activation(
                            pt.rearrange("p a b -> p (a b)"),
                            sps.rearrange("p a b -> p (a b)"),
                            FT.Exp, scale=SCALE)
                    else:
                        nc.scalar.activation(pt, sps, FT.Exp, scale=SCALE)
                    return pt

                # HAM promotion burst: ~4.5us of dependency-free back-to-back
                # matmuls. The p1 tail's stalls demote the PE clock to K=4/8;
                # promotion needs ~4us of continuous PE busy, which attention's
                # steady state never provides. This burst re-promotes once;
                # the per-step warm-keepers then hold K=8/8.
                burst = spsum.tile([128, 2, 512], F32, tag="sps")
                for _ in range(22):
                    nc.tensor.matmul(
                        burst[:, 0, :],
                        lhsT=QKall[:, 0:128], rhs=QKall[:, 0:512],
                        start=True, stop=True)

                ops = None
                deferred = []

                def consume(ops_t, pt_t, h, jt):
                    for ic in range(2):
                        nc.tensor.matmul(
                            ops_t[:, ic, :],
                            lhsT=vhs[h % 4][:, jt, :],
                            rhs=pt_t[:, ic, :],
                            start=(jt == 0), stop=(jt == 7),
                            skip_group_check=True)

                pt_next = produce(*steps[0])
                for s, (h, jt) in enumerate(steps):
                    po, hf = 64 * (h % 2), h // 2
                    if jt == 0:
                        ops = opsum.tile([128, 2, 512], F32, tag="ops")
                    ops_cur = ops
                    pt_cur = pt_next
                    if s + 1 < len(steps):
                        pt_next = produce(*steps[s + 1])
                    # DVE-offloaded steps' PV is deferred 2 steps so the PE's
                    # in-order queue never waits on the slower DVE exp chain.
                    deferred = [(d - 1, o, p, hh, jj)
                                for d, o, p, hh, jj in deferred]
                    while deferred and (deferred[0][0] <= 0 or jt == 7):
                        _, o, p, hh, jj = deferred.pop(0)
                        consume(o, p, hh, jj)
                    if exp_ops and jt in OFFLOAD_JT:
                        deferred.append((2, ops_cur, pt_cur, h, jt))
                    else:
                        consume(ops_cur, pt_cur, h, jt)
                    if jt == 5 and h + 3 < H:
                        load_v(h + 3)
                    if jt == 7:
                        # custom-DVE reciprocal can't read PSUM; stage the
                        # denominator through SBUF. One fused chain per head.
                        zst = rzpool.tile([64, 2, 512], F32, tag="zst")
                        nc.vector.tensor_copy(zst, ops_cur[64:128, :, :])
                        rzs = rzpool.tile([64, 2, 512], F32, tag="rzs")
                        nc.vector.reciprocal_approx_fast(rzs, zst)
                        nc.vector.tensor_mul(
                            oT[po:po + 64, hf, :, :],
                            ops_cur[0:64, :, :], rzs)

            # ---------- phase 3: out = oT.T @ w_out^T + b ----------
            with tc.tile_pool(name="p3st", bufs=4) as p3st, \
                 tc.tile_pool(name="ps3", bufs=4, space="PSUM") as ps3:
                for it in range(8):
                    for ec in range(2):
                        rps = ps3.tile([128, 512], F32)
                        for ct in range(8):
                            nc.tensor.matmul(
                                rps,
                                lhsT=oT[:, ct, it // 4,
                                        (it % 4) * 128:(it % 4 + 1) * 128],
                                rhs=WOT[:, ct, ec * 512:(ec + 1) * 512],
                                start=(ct == 0), stop=(ct == 7))
                        ost = p3st.tile([128, 512], F32)
                        nc.vector.tensor_add(
                            ost, rps, biasrep[:, ec * 512:(ec + 1) * 512])
                        nc.sync.dma_start(
                            out=outp[it * 128:(it + 1) * 128,
                                     ec * 512:(ec + 1) * 512],
                            in_=ost)

    nc.finalize()
    return nc


_CACHE = {}


def _get_nc():
    if "nc" not in _CACHE:
        _CACHE["nc"] = build()
    return _CACHE["nc"]


def make_in_maps(x, w_qkv, w_out, b_out):
    bf = ml_dtypes.bfloat16
    wqkvt = np.ascontiguousarray(np.asarray(w_qkv, dtype=np.float32).T).astype(bf)
    woutt = np.ascontiguousarray(np.asarray(w_out, dtype=np.float32).T).astype(bf)
    bias = np.ascontiguousarray(np.asarray(b_out, dtype=np.float32).reshape(1, DIM))
    x = np.asarray(x, dtype=np.float32)
    return [
        {
            "xt": np.ascontiguousarray(x[b].T).astype(bf),
            "wqkvt": wqkvt,
            "woutt": woutt,
            "bias": bias,
        }
        for b in range(N_CORES)
    ]


def kernel(x, w_qkv, w_out, b_out):
    nc = _get_nc()
    in_maps = make_in_maps(x, w_qkv, w_out, b_out)
    res = run_bass_kernel_spmd(nc, in_maps, core_ids=list(range(N_CORES)))
    return np.stack(
        [res.results[b]["out"] for b in range(N_CORES)], axis=0
    ).astype(np.float32)
